# revision 47
# baseline (speedup 1.0000x reference)
"""Trainium2 Bass kernel for nn_AutonomousAI_38182259262002.

Data-parallel over B across 8 cores. Each core computes the world model +
all-pairs causal scorer for its 4 batch rows; graph mean is done on host
during unsharding. Everything on device is feature-major ([128, ktile, b])
with host-prepared SBUF images so all DMAs are dense.
"""

import os
import numpy as np

B, OBS, D, V = 32, 768, 512, 64
NCORES = 8
BL = B // NCORES          # 4 batch rows per core
IJ = V * V                # 4096 (i,j) pairs
TWO_D = 2 * D             # 1024

_CACHE = {}


# ---------------------------------------------------------------- host images
def _fm_w(W, dtype=np.float32):
    """W [M, K] -> stationary (lhsT) image [128, K//128, M]."""
    M, K = W.shape
    return np.ascontiguousarray(
        W.T.reshape(K // 128, 128, M).transpose(1, 0, 2).astype(dtype)
    )


def _fm_x(x, dtype=np.float32):
    """x [N, F] -> feature-major moving image [128, F//128, N]."""
    N, F = x.shape
    return np.ascontiguousarray(
        x.T.reshape(F // 128, 128, N).transpose(1, 0, 2).astype(dtype)
    )


def _fm_b(v):
    """bias [F] -> [128, F//128]."""
    return np.ascontiguousarray(v.reshape(-1, 128).T)


def _split_multiwaits(nc, mybir, max_waits=1):
    """Walrus on this toolchain rejects instructions carrying more than one
    sync-wait command. Rewrite any multi-wait instruction into same-engine
    NOP prefixes that each carry a single wait."""
    n_split = 0
    for b in list(nc.m.functions[0].blocks):
        insts = list(b.instructions)
        new = []
        for inst in insts:
            si = inst.sync_info
            if si is not None and si.on_wait is not None and len(si.on_wait) > max_waits:
                waits = list(si.on_wait)
                ge = [w for w in waits if w.wait_mode == "sem-ge-imm"]
                other = [w for w in waits if w.wait_mode != "sem-ge-imm"]
                keep_n = max(0, max_waits - len(other))
                cut = len(ge) - keep_n
                move, keep = ge[:cut], other + ge[cut:]
                if len(keep) > max_waits:
                    move, keep = waits[:-max_waits], waits[-max_waits:]
                for j, w in enumerate(move):
                    nop = mybir.InstNoOp(name=f"{inst.name}-ws{j}")
                    nop.engine = inst.engine
                    nop.sync_info = mybir.SyncInfo(on_wait=[w], on_update=[])
                    new.append(nop)
                    n_split += 1
                inst.sync_info = mybir.SyncInfo(
                    on_wait=keep, on_update=list(si.on_update or [])
                )
            new.append(inst)
        b.instructions = new
    return n_split


# ---------------------------------------------------------------- bass program
def _build():
    if "nc" in _CACHE:
        return _CACHE["nc"]

    import concourse.bass as bass
    import concourse.tile as tile
    from concourse import mybir

    F32 = mybir.dt.float32
    F16 = mybir.dt.float16
    AF = mybir.ActivationFunctionType
    OP = mybir.AluOpType

    nc = bass.Bass(target_bir_lowering=False)

    EI, EO = "ExternalInput", "ExternalOutput"
    # fp16 inputs: weights on the critical DMA path + matmul moving operands.
    # All small constants travel in one packed [128, 160] f32 tensor so the
    # head pays a single dma_start issue cost instead of ~15.
    d_cpack = nc.dram_tensor("cpack", [128, 224], F32, kind=EI)
    d_waT = nc.dram_tensor("waT", [128, 4, TWO_D], F16, kind=EI)
    d_wbT = nc.dram_tensor("wbT", [128, 4, TWO_D], F16, kind=EI)
    d_wcT = nc.dram_tensor("wcT", [128, 4, TWO_D], F16, kind=EI)
    d_w1T = nc.dram_tensor("w1T", [128, 6, TWO_D], F16, kind=EI)
    d_w2eT = nc.dram_tensor("w2eT", [128, 8, D], F16, kind=EI)
    d_wihT = nc.dram_tensor("wihT", [128, 8, 3 * D], F16, kind=EI)
    d_decT = nc.dram_tensor("decT", [128, 4, OBS], F32, kind=EI)
    d_rs1T = nc.dram_tensor("rs1T", [128, 4, D], F32, kind=EI)
    d_rs2T = nc.dram_tensor("rs2T", [128, 4, D], F32, kind=EI)

    d_hT = nc.dram_tensor("hT_o", [128, 4, BL], F32, kind=EO)
    d_predT = nc.dram_tensor("predT_o", [128, 6, BL], F32, kind=EO)
    d_rsT = nc.dram_tensor("rsT_o", [128, 4, BL], F32, kind=EO)
    d_sc = nc.dram_tensor("sc_o", [128, 128], F32, kind=EO)

    RSQRT2 = 0.7071067811865476

    with tile.TileContext(nc) as tc:
        with (
            tc.tile_pool(name="consts", bufs=1) as cp,
            tc.tile_pool(name="weights", bufs=3) as wp,
            tc.tile_pool(name="spool", bufs=1) as sp,
            tc.tile_pool(name="gpool", bufs=4) as gp,
            tc.tile_pool(name="prepool", bufs=8) as pp,
        ):
            # ---------------- packed constants + enc1 weights (critical path)
            consts_t = cp.tile([128, 224], F32)
            nc.gpsimd.dma_start(out=consts_t, in_=d_cpack[:, :])
            obs_t = consts_t[:, 0:12].bitcast(F16).rearrange(
                "p (a b) -> p a b", a=6
            )
            emb_t = consts_t[:, 12:140].bitcast(F16).rearrange(
                "p (a b) -> p a b", a=4
            )
            w2h_t = consts_t[:, 140:144].bitcast(F16)
            act_t = consts_t[:, 144:160].rearrange("p (a b) -> p a b", a=4)
            eb1_t = consts_t[:, 160:168]
            eb2_t = consts_t[:, 168:172]
            bih_t = consts_t[:, 172:184]
            bhh_t = consts_t[:, 184:196]
            db_t = consts_t[:, 196:202]
            rb1_t = consts_t[:, 202:206]
            rb2_t = consts_t[:, 206:210]
            scb1_t = consts_t[:, 210:218]
            scb2_t = consts_t[:, 218:219]
            id4_t = consts_t[0:BL, 219:223]
            w1full = wp.tile([128, 6, TWO_D], F16, tag="w1", name="w1full")
            nc.sync.dma_start(out=w1full, in_=d_w1T[:, :, :])

            ea_t = cp.tile([128, 8, V], F32)
            eb_t = cp.tile([128, 8, V], F32)

            with tc.tile_pool(name="psA", bufs=2, space="PSUM") as psA:
                # ---------------- PE warmup: keep the HAM clock-gate busy with
                # tiny matmuls while the first weights stream in, so the real
                # head matmuls run at 2.4 GHz instead of 1.2.
                warm_ps = psA.tile([BL, BL], F32, tag="wm", name="warm_ps")
                for _ in range(80):
                    nc.tensor.matmul(warm_ps, id4_t, id4_t, start=True, stop=True)

                # ---------------- enc layer 1: a1 = obs @ enc_w1.T   [1024, BL]
                a1_ps = psA.tile([128, 8, BL], F32, tag="wm", name="a1_ps")
                for mt in range(8):
                    for kt in range(6):
                        nc.tensor.matmul(
                            a1_ps[:, mt, :],
                            w1full[:, kt, mt * 128 : (mt + 1) * 128],
                            obs_t[:, kt, :],
                            start=(kt == 0),
                            stop=(kt == 5),
                        )
                # exact gelu via erf (sigmoid table set):
                # g1 = 0.5*(a1+b1)*(1 + erf((a1+b1)/sqrt(2)))
                a1b_t = cp.tile([128, 8, BL], F32)
                nc.vector.tensor_tensor(
                    a1b_t, a1_ps,
                    eb1_t[:, :, None].to_broadcast((128, 8, BL)), OP.add,
                )
                x05_t = cp.tile([128, 8, BL], F32)
                erf_t = cp.tile([128, 8, BL], F32)
                g1_t = cp.tile([128, 8, BL], F16)
                nc.scalar.activation(x05_t, a1b_t, AF.Identity, scale=0.5)
                nc.scalar.activation(erf_t, a1b_t, AF.Erf, scale=RSQRT2)
                nc.vector.tensor_mul(g1_t, x05_t, erf_t)
                nc.vector.tensor_add(g1_t, g1_t, x05_t)

                # ---------------- enc layer 2 -> z, packed into x = [z; action]
                x_t = cp.tile([128, 8, BL], F16)
                w2e = wp.tile([128, 8, D], F16, tag="w", name="w2e")
                nc.sync.dma_start(out=w2e, in_=d_w2eT[:, :, :])
                z_ps = psA.tile([128, 4, BL], F32, tag="wm", name="z_ps")
                for mt in range(4):
                    for kt in range(8):
                        nc.tensor.matmul(
                            z_ps[:, mt, :],
                            w2e[:, kt, mt * 128 : (mt + 1) * 128],
                            g1_t[:, kt, :],
                            start=(kt == 0),
                            stop=(kt == 7),
                        )
                nc.vector.tensor_tensor(
                    x_t[:, 0:4, :], z_ps,
                    eb2_t[:, :, None].to_broadcast((128, 4, BL)), OP.add,
                )
                nc.vector.tensor_copy(x_t[:, 4:8, :], act_t)

                # ---------------- GRU (h0 = 0): gi = wih @ x + bih + bhh
                bb_t = cp.tile([128, 12], F32)
                nc.vector.tensor_add(bb_t, bih_t, bhh_t)
                # gi batch-major with x stationary (8 tiny LDWs instead of 96
                # full ones), then PE-transpose back to feature-major. Each
                # 512 chunk accumulates over k in its own PSUM bank.
                gi_bm = psA.tile([BL, 3 * D], F32, tag="wm", name="gi_bm")
                for ci in range(4):
                    wih_c = wp.tile([128, 2, 3 * D], F16, tag="w", name=f"wih{ci}")
                    nc.sync.dma_start(out=wih_c, in_=d_wihT[:, 2 * ci : 2 * ci + 2, :])
                    for kt in range(2):
                        gk = 2 * ci + kt
                        for nch in range(3):
                            nc.tensor.matmul(
                                gi_bm[:, nch * 512 : (nch + 1) * 512],
                                x_t[:, gk, :],
                                wih_c[:, kt, nch * 512 : (nch + 1) * 512],
                                start=(gk == 0),
                                stop=(gk == 7),
                            )
                w_ab = {}
                for wnm, d_w in (("wa", d_waT), ("wb", d_wbT)):
                    w_ab[wnm] = wp.tile(
                        [128, 4, TWO_D], F16, tag="w", name=f"w_{wnm}"
                    )
                    nc.sync.dma_start(out=w_ab[wnm], in_=d_w[:, :, :])
                wc = wp.tile([128, 4, TWO_D], F16, tag="w", name="wc")
                nc.sync.dma_start(out=wc, in_=d_wcT[:, :, :])
                gi_sbm = cp.tile([BL, 3 * D], F32)
                nc.vector.tensor_copy(gi_sbm, gi_bm)
                gi_ps = psA.tile([128, 12, BL], F32, tag="wm", name="gi_fm")
                for mt in range(12):
                    nc.tensor.transpose(
                        gi_ps[:, mt, :],
                        gi_sbm[:, mt * 128 : (mt + 1) * 128],
                        id4_t,
                    )
                # ru = sigmoid(gi[:, 0:8] + bih + bhh), one wide ACT
                gb_t = cp.tile([128, 8, BL], F32)
                nc.vector.tensor_tensor(
                    gb_t, gi_ps[:, 0:8, :],
                    bb_t[:, 0:8, None].to_broadcast((128, 8, BL)), OP.add,
                )
                ru_t = cp.tile([128, 8, BL], F32)
                nc.scalar.activation(ru_t, gb_t, AF.Sigmoid)
                # n = tanh(gi_n + bih_n + r * bhh_n)
                n_t = cp.tile([128, 4, BL], F32)
                tmp_t = cp.tile([128, 4, BL], F32)
                nc.vector.tensor_tensor(
                    tmp_t, ru_t[:, 0:4, :],
                    bhh_t[:, 8:12, None].to_broadcast((128, 4, BL)), OP.mult,
                )
                nc.vector.tensor_add(tmp_t, tmp_t, gi_ps[:, 8:12, :])
                nc.vector.tensor_tensor(
                    tmp_t, tmp_t,
                    bih_t[:, 8:12, None].to_broadcast((128, 4, BL)), OP.add,
                )
                nc.scalar.activation(n_t, tmp_t, AF.Tanh)
                # dummy gelu tied to n_t: pulls the gelu table load off the
                # critical path (otherwise it serializes before the first
                # big gelu, behind the late reasoner-weight DMA)
                warm_gelu = cp.tile([1, 1], F32)
                nc.scalar.activation(warm_gelu, n_t[0:1, 0, 0:1], AF.Gelu)
                # h = (1 - u) * n
                h_t = cp.tile([128, 4, BL], F32)
                nc.vector.tensor_mul(h_t, ru_t[:, 4:8, :], n_t)
                nc.vector.tensor_tensor(h_t, n_t, h_t, OP.subtract)
                nc.sync.dma_start(out=d_hT[:, :, :], in_=h_t)

                # ---------------- Cc = Wc @ (h + action), V bias = Cc + sc_b1
                ha_t = cp.tile([128, 4, BL], F16)
                nc.vector.tensor_add(ha_t, h_t, act_t)
                cc_ps = psA.tile([128, 8, BL], F32, tag="wm", name="cc_ps")
                for mt in range(8):
                    for kt in range(4):
                        nc.tensor.matmul(
                            cc_ps[:, mt, :],
                            wc[:, kt, mt * 128 : (mt + 1) * 128],
                            ha_t[:, kt, :],
                            start=(kt == 0),
                            stop=(kt == 3),
                        )
                vt_t = cp.tile([128, 8, BL], F32)
                nc.vector.tensor_tensor(
                    vt_t, cc_ps,
                    scb1_t[:, :, None].to_broadcast((128, 8, BL)), OP.add,
                )

                # ---------------- Ea / Eb = embed @ Wa.T / Wb.T (feature-major)
                for w, out_t in ((w_ab["wa"], ea_t), (w_ab["wb"], eb_t)):
                    for mt in range(8):
                        ps = psA.tile([128, V], F32, tag="ea", name="ea_ps")
                        for kt in range(4):
                            nc.tensor.matmul(
                                ps,
                                w[:, kt, mt * 128 : (mt + 1) * 128],
                                emb_t[:, kt, :],
                                start=(kt == 0),
                                stop=(kt == 3),
                            )
                        nc.vector.tensor_copy(out_t[:, mt, :], ps)

                # ---------------- decoder: pred = h @ dec_w.T + dec_b
                dec_w = wp.tile([128, 4, OBS], F32, tag="w", name="dec_w")
                nc.sync.dma_start(out=dec_w, in_=d_decT[:, :, :])
                dec_ps = psA.tile([128, 6, BL], F32, tag="wm", name="dec_ps")
                for mt in range(6):
                    for kt in range(4):
                        nc.tensor.matmul(
                            dec_ps[:, mt, :],
                            dec_w[:, kt, mt * 128 : (mt + 1) * 128],
                            h_t[:, kt, :],
                            start=(kt == 0),
                            stop=(kt == 3),
                        )
                pred_t = cp.tile([128, 6, BL], F32)
                nc.vector.tensor_tensor(
                    pred_t, dec_ps,
                    db_t[:, :, None].to_broadcast((128, 6, BL)), OP.add,
                )
                nc.sync.dma_start(out=d_predT[:, :, :], in_=pred_t)

                # ---------------- reasoner (uses the gelu table set)
                rs1_w = wp.tile([128, 4, D], F32, tag="w", name="rs1_w")
                nc.sync.dma_start(out=rs1_w, in_=d_rs1T[:, :, :])
                rs1_ps = psA.tile([128, 4, BL], F32, tag="wm", name="rs1_ps")
                for mt in range(4):
                    for kt in range(4):
                        nc.tensor.matmul(
                            rs1_ps[:, mt, :],
                            rs1_w[:, kt, mt * 128 : (mt + 1) * 128],
                            h_t[:, kt, :],
                            start=(kt == 0),
                            stop=(kt == 3),
                        )
                rg1_t = cp.tile([128, 4, BL], F32)
                nc.vector.tensor_tensor(
                    rg1_t, rs1_ps,
                    rb1_t[:, :, None].to_broadcast((128, 4, BL)), OP.add,
                )
                rg_t = cp.tile([128, 4, BL], F32)
                nc.scalar.activation(rg_t, rg1_t, AF.Gelu)
                rs2_w = wp.tile([128, 4, D], F32, tag="w", name="rs2_w")
                nc.sync.dma_start(out=rs2_w, in_=d_rs2T[:, :, :])
                rs2_ps = psA.tile([128, 4, BL], F32, tag="wm", name="rs2_ps")
                for mt in range(4):
                    for kt in range(4):
                        nc.tensor.matmul(
                            rs2_ps[:, mt, :],
                            rs2_w[:, kt, mt * 128 : (mt + 1) * 128],
                            rg_t[:, kt, :],
                            start=(kt == 0),
                            stop=(kt == 3),
                        )
                rs_t = cp.tile([128, 4, BL], F32)
                nc.vector.tensor_tensor(
                    rs_t, rs2_ps,
                    rb2_t[:, :, None].to_broadcast((128, 4, BL)), OP.add,
                )
                nc.sync.dma_start(out=d_rsT[:, :, :], in_=rs_t)

            # ---------------- all-pairs causal scorer
            # scores[b, i, j] = sigmoid(w2 . gelu(Cc[b] + Ea[i] + Eb[j] + b1))
            # S_k[d, i, j] = EaT + EbT (built once), gelu bias = Cc + b1 per b.
            # PE reduces over d (partitions) with stationary w2[128, 1]; each
            # 512-wide ij chunk accumulates over k in its own PSUM bank.
            sc128_t = cp.tile([128, 128], F16)
            sc4o_t = cp.tile([128, 128], F32)
            sk = [None] * 8
            with tc.tile_pool(name="psB", bufs=1, space="PSUM") as psB:
                for b in range(BL):
                    ps_c = [
                        psB.tile([1, 512], F32, tag=f"big{c}", name=f"ps{b}_{c}")
                        for c in range(8)
                    ]
                    for k in range(8):
                        if b == 0:
                            # split the S_k build across DVE and GpSimd so it
                            # keeps ahead of the gelu consumer on ScalarE
                            sk[k] = sp.tile(
                                [128, V, V], F16, tag=f"s{k}", name=f"s{k}"
                            )
                            for r0, r1 in ((0, 19), (19, 38)):
                                nc.vector.tensor_tensor(
                                    sk[k][:, r0:r1, :],
                                    ea_t[:, k, r0:r1, None].to_broadcast(
                                        (128, r1 - r0, V)
                                    ),
                                    eb_t[:, k : k + 1, :].to_broadcast(
                                        (128, r1 - r0, V)
                                    ),
                                    OP.add,
                                )
                            nc.gpsimd.tensor_tensor(
                                sk[k][:, 38:V, :],
                                ea_t[:, k, 38:V, None].to_broadcast((128, 26, V)),
                                eb_t[:, k : k + 1, :].to_broadcast((128, 26, V)),
                                OP.add,
                            )
                        g_t = gp.tile([128, V, V], F16, tag="g", name="g_t")
                        last_g = g_t
                        nc.scalar.activation(
                            g_t, sk[k], AF.Gelu, bias=vt_t[:, k, b : b + 1]
                        )
                        for c in range(8):
                            nc.tensor.matmul(
                                ps_c[c],
                                w2h_t[:, k : k + 1],
                                g_t[:, c * 8 : (c + 1) * 8, :],
                                start=(k == 0),
                                stop=(k == 7),
                            )
                    for c in range(8):
                        pre_t = pp.tile([1, 512], F16, tag="pre", name="pre_t")
                        nc.vector.tensor_copy(pre_t, ps_c[c])
                        nc.sync.dma_start(
                            out=sc128_t[32 * b + 4 * c : 32 * b + 4 * c + 4, :],
                            in_=pre_t.rearrange("p (a f) -> p a f", a=4),
                        )

            # dummy sigmoid tied to the last gelu output: forces the
            # sigmoid table load to overlap the score copies/gathers instead
            # of serializing before the real sigmoid
            warm_sig = cp.tile([1, 1], F32)
            nc.scalar.activation(warm_sig, last_g[0:1, 0, 0:1], AF.Sigmoid)
            nc.scalar.activation(sc4o_t, sc128_t, AF.Sigmoid, bias=scb2_t)
            nc.sync.dma_start(out=d_sc[:, :], in_=sc4o_t)

    _split_multiwaits(nc, mybir)
    _CACHE["nc"] = nc
    return nc


# ---------------------------------------------------------------- entry point
def _prepare_in_maps(inputs):
    f32 = lambda a: np.ascontiguousarray(np.asarray(a), dtype=np.float32)
    obs = f32(inputs["obs"])
    action = f32(inputs["action"])
    embed = f32(inputs["embed"])
    sc_w1 = f32(inputs["sc_w1"])
    f16 = np.float16

    def pack16(a):
        return np.ascontiguousarray(a.astype(np.float16)).reshape(128, -1).view(np.float32)

    cpack = np.zeros((128, 224), np.float32)
    cpack[:, 12:140] = pack16(_fm_x(embed, f16))
    cpack[:, 140:144] = pack16(
        f32(inputs["sc_w2"])[0].reshape(8, 128).T.astype(np.float16)
    )
    cpack[:, 160:168] = _fm_b(f32(inputs["enc_b1"]))
    cpack[:, 168:172] = _fm_b(f32(inputs["enc_b2"]))
    cpack[:, 172:184] = _fm_b(f32(inputs["gru_bih"]))
    cpack[:, 184:196] = _fm_b(f32(inputs["gru_bhh"]))
    cpack[:, 196:202] = _fm_b(f32(inputs["dec_b"]))
    cpack[:, 202:206] = _fm_b(f32(inputs["rs_b1"]))
    cpack[:, 206:210] = _fm_b(f32(inputs["rs_b2"]))
    cpack[:, 210:218] = _fm_b(f32(inputs["sc_b1"]))
    cpack[:, 218] = float(np.asarray(inputs["sc_b2"])[0])
    cpack[0:BL, 219:223] = np.eye(BL, dtype=np.float32)

    common = {
        "waT": _fm_w(sc_w1[:, :D], f16),
        "wbT": _fm_w(sc_w1[:, D : 2 * D], f16),
        "wcT": _fm_w(sc_w1[:, 2 * D :], f16),
        "w1T": _fm_w(f32(inputs["enc_w1"]), f16),
        "w2eT": _fm_w(f32(inputs["enc_w2"]), f16),
        "wihT": _fm_w(f32(inputs["gru_wih"]), f16),
        "decT": _fm_w(f32(inputs["dec_w"])),
        "rs1T": _fm_w(f32(inputs["rs_w1"])),
        "rs2T": _fm_w(f32(inputs["rs_w2"])),
    }
    in_maps = []
    for c in range(NCORES):
        m = dict(common)
        cp_c = cpack.copy()
        cp_c[:, 0:12] = pack16(_fm_x(obs[c * BL : (c + 1) * BL], f16))
        cp_c[:, 144:160] = _fm_x(action[c * BL : (c + 1) * BL]).reshape(128, 16)
        m["cpack"] = cp_c
        in_maps.append(m)
    return in_maps


def kernel(**inputs):
    from concourse.bass_utils import run_bass_kernel_spmd

    nc = _build()
    in_maps = _prepare_in_maps(inputs)

    trace = os.environ.get("KERNEL_TRACE", "0") == "1"
    try:
        res = run_bass_kernel_spmd(
            nc, in_maps, core_ids=list(range(NCORES)), trace=trace
        )
    except (ImportError, ModuleNotFoundError):
        # NTFF profiling hook unavailable (e.g. axon client w/o antenv hooks)
        res = run_bass_kernel_spmd(nc, in_maps, core_ids=list(range(NCORES)))
    if trace and res.exec_time_ns is not None:
        print(f"HW exec time: {res.exec_time_ns} ns")
        if res.instructions_and_trace is not None:
            print(f"trace: {res.instructions_and_trace[1]}")
    results = res.results

    h = np.concatenate(
        [r["hT_o"].transpose(2, 1, 0).reshape(BL, D) for r in results], axis=0
    )
    pred = np.concatenate(
        [r["predT_o"].transpose(2, 1, 0).reshape(BL, OBS) for r in results], axis=0
    )
    reasoning = np.concatenate(
        [r["rsT_o"].transpose(2, 1, 0).reshape(BL, D) for r in results], axis=0
    )
    scores = np.concatenate(
        [r["sc_o"].reshape(BL, IJ) for r in results], axis=0
    ).reshape(B, V, V)
    graph = scores.mean(axis=0)
    return (
        h.astype(np.float32),
        pred.astype(np.float32),
        graph.astype(np.float32),
        reasoning.astype(np.float32),
    )
